# revision 4
# baseline (speedup 1.0000x reference)
"""Trainium2 Bass kernel for dense-MoE forward (nn_MoE_32238024524134).

Reference computation (per token b of B=8192):
  scores = softmax(relu(x@gw1+gb1) @ gw2 + gb2)            [B, E]
  h      = relu(einsum('bd,edh', x, W1) + b1)              [B, E, H]
  out    = einsum('beh,ehk', h, W2) + b2                   [B, E, H]
  logits = einsum('beh,hc', out, clf_w) + clf_b            [B, E, C]

Sharding: expert-parallel. Core e computes expert e's full chain for ALL
tokens; the gate (router softmax) is data-parallel - core e computes the
scores for its 1024-token slice. Host assembles full outputs.

On-chip layout: activations kept transposed (feature on partitions), so
every GEMM contracts along partitions with naturally-laid-out weights as
the stationary operand. x is pre-transposed and pre-cast to fp16 on host.
Matmuls run in fp16 (1 cycle/row on PE - same speed as bf16) with fp32
PSUM accumulation; measured end-to-end L2 rel err vs fp32 ~5e-4.
"""

import os
import numpy as np
from contextlib import ExitStack

D, H, E, C, B = 1024, 2048, 8, 1024, 8192
NCORES = 8
BT = B // NCORES          # tokens per core for the gate phase
NB = 512                  # moving free dim (tokens per matmul)
NBT = B // NB             # number of b-tiles in the expert phase
P = 128

# Stash of the last BassKernelResults (for test.py to read exec_time_ns).
LAST_RESULT = None
_CACHED_NC = None


def _build_nc():
    import concourse.bass as bass
    import concourse.bacc as bacc
    import concourse.tile as tile
    from concourse import mybir

    ts = bass.ts
    f16 = mybir.dt.float16
    f32 = mybir.dt.float32
    AF = mybir.ActivationFunctionType

    nc = bacc.Bacc(None, target_bir_lowering=False)

    # ---- DRAM I/O ----
    xT = nc.dram_tensor("xT", [D, B], f16, kind="ExternalInput")
    w1 = nc.dram_tensor("w1", [D, H], f16, kind="ExternalInput")
    w2 = nc.dram_tensor("w2", [H, H], f16, kind="ExternalInput")
    clf = nc.dram_tensor("clf", [H, C], f16, kind="ExternalInput")
    gw1 = nc.dram_tensor("gw1", [D, D], f16, kind="ExternalInput")
    gw2 = nc.dram_tensor("gw2", [D, E], f16, kind="ExternalInput")
    xg = nc.dram_tensor("xg", [D, BT], f16, kind="ExternalInput")
    b1 = nc.dram_tensor("b1", [P, H // P], f32, kind="ExternalInput")
    b2 = nc.dram_tensor("b2", [P, H // P], f32, kind="ExternalInput")
    clfb = nc.dram_tensor("clfb", [P, C // P], f32, kind="ExternalInput")
    gb1 = nc.dram_tensor("gb1", [P, D // P], f32, kind="ExternalInput")
    gb2 = nc.dram_tensor("gb2", [P, E], f32, kind="ExternalInput")
    logits = nc.dram_tensor("logits", [C, B], f32, kind="ExternalOutput")
    scores = nc.dram_tensor("scores", [BT, E], f32, kind="ExternalOutput")

    KD = D // P    # 8  k-subtiles for D contraction
    KH = H // P    # 16 k-subtiles for H contraction
    MH = H // P    # 16 output tiles for H
    MC = C // P    # 8  output tiles for C

    xT_r = xT[:].rearrange("(a p) b -> p a b", p=P)      # [128, 8, B]
    w1_r = w1[:].rearrange("(a p) h -> p a h", p=P)      # [128, 8, H]
    w2_r = w2[:].rearrange("(a p) h -> p a h", p=P)      # [128, 16, H]
    clf_r = clf[:].rearrange("(a p) c -> p a c", p=P)    # [128, 16, C]
    gw1_r = gw1[:].rearrange("(a p) d -> p a d", p=P)    # [128, 8, D]
    gw2_r = gw2[:].rearrange("(a p) e -> p a e", p=P)    # [128, 8, E]
    xg_r = xg[:].rearrange("(a p) t -> p a t", p=P)      # [128, 8, BT]

    with tile.TileContext(nc) as tc:
        with ExitStack() as ctx:
            const = ctx.enter_context(tc.tile_pool(name="const", bufs=1))
            wpool = ctx.enter_context(tc.tile_pool(name="weights", bufs=1))
            xpool = ctx.enter_context(tc.tile_pool(name="x", bufs=2))
            lpool = ctx.enter_context(tc.tile_pool(name="lg", bufs=4))
            psum = ctx.enter_context(tc.tile_pool(name="psum", bufs=4, space="PSUM"))

            # ---- biases (tiny, loaded once) ----
            b1_t = const.tile([P, H // P], f32, name="b1t")
            nc.sync.dma_start(b1_t[:], b1[:])
            b2_t = const.tile([P, H // P], f32, name="b2t")
            nc.sync.dma_start(b2_t[:], b2[:])
            clfb_t = const.tile([P, C // P], f32, name="clfbt")
            nc.sync.dma_start(clfb_t[:], clfb[:])
            gb1_t = const.tile([P, D // P], f32, name="gb1t")
            nc.sync.dma_start(gb1_t[:], gb1[:])
            gb2_t = const.tile([P, E], f32, name="gb2t")
            nc.sync.dma_start(gb2_t[:], gb2[:])

            # ---- gate phase (own pools; space reused by h/o pools after) ----
            with ExitStack() as gctx:
                gpool = gctx.enter_context(tc.tile_pool(name="gate", bufs=1))
                spool = gctx.enter_context(tc.tile_pool(name="sm", bufs=4))

                gw1_t = gpool.tile([P, KD, D], f16, name="gw1t")
                for a in range(KD):
                    nc.sync.dma_start(gw1_t[:, a : a + 1, :], gw1_r[:, a : a + 1, :])
                xg_t = gpool.tile([P, KD, BT], f16, name="xgt")
                for a in range(KD):
                    nc.sync.dma_start(xg_t[:, a : a + 1, :], xg_r[:, a : a + 1, :])
                gw2_t = gpool.tile([P, KD, E], f16, name="gw2t")
                nc.sync.dma_start(gw2_t[:], gw2_r[:])

                # expert weights + first x tile: issue DMAs now so they run
                # during gate compute (no deps -> DMA engines start at once)
                w1_t = wpool.tile([P, KD, H], f16, name="w1t")
                for a in range(KD):
                    nc.sync.dma_start(w1_t[:, a : a + 1, :], w1_r[:, a : a + 1, :])
                x_tiles = [None] * NBT
                x_tiles[0] = xpool.tile([P, KD, NB], f16, name="xt")
                for a in range(KD):
                    nc.sync.dma_start(
                        x_tiles[0][:, a : a + 1, :], xT_r[:, a : a + 1, ts(0, NB)]
                    )
                w2_t = wpool.tile([P, KH, H], f16, name="w2t")
                for a in range(KH):
                    nc.sync.dma_start(w2_t[:, a : a + 1, :], w2_r[:, a : a + 1, :])
                clf_t = wpool.tile([P, KH, C], f16, name="clft")
                for a in range(KH):
                    nc.sync.dma_start(clf_t[:, a : a + 1, :], clf_r[:, a : a + 1, :])

                # gate stage 1: gT = relu(gw1.T @ xg + gb1)   [D, BT]
                g_t = gpool.tile([P, KD, BT], f16, name="gt")
                for m in range(KD):
                    for n in range(BT // NB):
                        ps = psum.tile([P, NB], f32, name="ps")
                        for k in range(KD):
                            nc.tensor.matmul(
                                ps[:],
                                gw1_t[:, k : k + 1, ts(m, P)],
                                xg_t[:, k : k + 1, ts(n, NB)],
                                start=(k == 0),
                                stop=(k == KD - 1),
                            )
                        nc.scalar.activation(
                            g_t[:, m, ts(n, NB)], ps[:], AF.Relu,
                            bias=gb1_t[:, m : m + 1],
                        )

                # gate stage 2: s = softmax(gT.T @ gw2 + gb2) [BT, E]
                for t in range(BT // P):
                    ps = psum.tile([P, E], f32, name="pss", bufs=2)
                    for k in range(KD):
                        nc.tensor.matmul(
                            ps[:],
                            g_t[:, k : k + 1, ts(t, P)],
                            gw2_t[:, k : k + 1, :],
                            start=(k == 0),
                            stop=(k == KD - 1),
                        )
                    sl = spool.tile([P, E], f32, name="sl")
                    nc.vector.tensor_add(sl[:], ps[:], gb2_t[:])
                    ex = spool.tile([P, E], f32, name="ex")
                    nc.scalar.activation(ex[:], sl[:], AF.Exp)
                    sm = spool.tile([P, 1], f32, name="smr")
                    nc.vector.tensor_reduce(
                        sm[:], ex[:], axis=mybir.AxisListType.X, op=mybir.AluOpType.add
                    )
                    rs = spool.tile([P, 1], f32, name="rsr")
                    nc.vector.reciprocal(rs[:], sm[:])
                    so = spool.tile([P, E], f32, name="so")
                    nc.vector.tensor_scalar_mul(so[:], ex[:], rs[:])
                    nc.sync.dma_start(scores[ts(t, P), :], so[:])

            # ---- expert phase: fused 3-GEMM chain per b-tile ----
            hpool = ctx.enter_context(tc.tile_pool(name="h", bufs=1))
            opool = ctx.enter_context(tc.tile_pool(name="o", bufs=1))

            for b in range(NBT):
                x_t = x_tiles[b]
                # prefetch next b-tile of x
                if b + 1 < NBT:
                    x_tiles[b + 1] = xpool.tile([P, KD, NB], f16, name="xt")
                    for a in range(KD):
                        nc.sync.dma_start(
                            x_tiles[b + 1][:, a : a + 1, :],
                            xT_r[:, a : a + 1, ts(b + 1, NB)],
                        )

                # S1: hT = relu(W1.T @ xT + b1)    [H, NB]
                h_t = hpool.tile([P, MH, NB], f16, name="ht")
                for m in range(MH):
                    ps = psum.tile([P, NB], f32, name="ps")
                    for k in range(KD):
                        nc.tensor.matmul(
                            ps[:],
                            w1_t[:, k : k + 1, ts(m, P)],
                            x_t[:, k : k + 1, :],
                            start=(k == 0),
                            stop=(k == KD - 1),
                        )
                    nc.scalar.activation(
                        h_t[:, m, :], ps[:], AF.Relu, bias=b1_t[:, m : m + 1]
                    )

                # S2: oT = W2.T @ hT + b2          [H, NB]
                o_t = opool.tile([P, MH, NB], f16, name="ot")
                for m in range(MH):
                    ps = psum.tile([P, NB], f32, name="ps")
                    for k in range(KH):
                        nc.tensor.matmul(
                            ps[:],
                            w2_t[:, k : k + 1, ts(m, P)],
                            h_t[:, k : k + 1, :],
                            start=(k == 0),
                            stop=(k == KH - 1),
                        )
                    nc.vector.tensor_scalar_add(o_t[:, m, :], ps[:], b2_t[:, m : m + 1])

                # S3: logitsT = clf.T @ oT + clfb  [C, NB]
                for m in range(MC):
                    ps = psum.tile([P, NB], f32, name="ps")
                    for k in range(KH):
                        nc.tensor.matmul(
                            ps[:],
                            clf_t[:, k : k + 1, ts(m, P)],
                            o_t[:, k : k + 1, :],
                            start=(k == 0),
                            stop=(k == KH - 1),
                        )
                    lg = lpool.tile([P, NB], f32, name="lg")
                    nc.vector.tensor_scalar_add(lg[:], ps[:], clfb_t[:, m : m + 1])
                    nc.sync.dma_start(logits[ts(m, P), ts(b, NB)], lg[:])

    nc.finalize()
    return nc


def _run_spmd_timed(nc, in_maps, n_cores, n_iters):
    """Mirror of bass2jax.run_bass_via_pjrt's multi-core path WITHOUT buffer
    donation, so the jitted NEFF can be re-executed on device-resident arrays
    to wall-clock the execution. Only used for perf measurement (test.py);
    grading path uses run_bass_kernel_spmd."""
    import time
    import jax
    from jax.experimental.shard_map import shard_map
    from jax.sharding import Mesh, PartitionSpec
    from concourse import mybir
    from concourse.bass2jax import (
        _bass_exec_p,
        install_neuronx_cc_hook,
        partition_id_tensor,
    )

    install_neuronx_cc_hook()
    partition_name = nc.partition_id_tensor.name if nc.partition_id_tensor else None

    in_names, out_names, out_avals, zero_outs = [], [], [], []
    for alloc in nc.m.functions[0].allocations:
        if not isinstance(alloc, mybir.MemoryLocationSet):
            continue
        name = alloc.memorylocations[0].name
        if alloc.kind == "ExternalInput":
            if name != partition_name:
                in_names.append(name)
        elif alloc.kind == "ExternalOutput":
            out_names.append(name)
            shape = tuple(alloc.tensor_shape)
            dtype = mybir.dt.np(alloc.dtype)
            out_avals.append(jax.core.ShapedArray(shape, dtype))
            zero_outs.append(np.zeros(shape, dtype))
    n_params = len(in_names)
    all_in_names = list(in_names) + list(out_names)
    if partition_name is not None:
        all_in_names.append(partition_name)

    def _body(*args):
        operands = list(args)
        if partition_name is not None:
            operands.append(partition_id_tensor())
        outs = _bass_exec_p.bind(
            *operands,
            out_avals=tuple(out_avals),
            in_names=tuple(all_in_names),
            out_names=tuple(out_names),
            lowering_input_output_aliases=(),
            sim_require_finite=True,
            sim_require_nnan=True,
            nc=nc,
        )
        return tuple(outs)

    devices = jax.devices()[:n_cores]
    mesh = Mesh(np.asarray(devices), ("core",))
    n_outs = len(out_names)
    sharded = jax.jit(
        shard_map(
            _body,
            mesh=mesh,
            in_specs=(PartitionSpec("core"),) * (n_params + n_outs),
            out_specs=(PartitionSpec("core"),) * n_outs,
            check_rep=False,
        ),
        keep_unused=True,
    )
    concat_in = [
        np.concatenate([np.asarray(m[name]) for m in in_maps], axis=0)
        for name in in_names
    ]
    concat_zeros = [
        np.zeros((n_cores * z.shape[0], *z.shape[1:]), z.dtype) for z in zero_outs
    ]
    sh = jax.sharding.NamedSharding(mesh, PartitionSpec("core"))
    dev_args = [jax.device_put(a, sh) for a in concat_in + concat_zeros]
    out_arrs = jax.block_until_ready(sharded(*dev_args))  # compile + 1st exec

    times = []
    for _ in range(n_iters):
        t0 = time.perf_counter()
        jax.block_until_ready(sharded(*dev_args))
        times.append(time.perf_counter() - t0)

    results = [
        {
            name: np.asarray(out_arrs[i]).reshape(n_cores, *out_avals[i].shape)[c]
            for i, name in enumerate(out_names)
        }
        for c in range(n_cores)
    ]
    return results, times


def kernel(**inputs):
    global LAST_RESULT, _CACHED_NC
    from concourse.bass_utils import run_bass_kernel_spmd

    x = np.asarray(inputs["x"], dtype=np.float32)
    W1 = np.asarray(inputs["W1"], dtype=np.float32)
    b1 = np.asarray(inputs["b1"], dtype=np.float32)
    W2 = np.asarray(inputs["W2"], dtype=np.float32)
    b2 = np.asarray(inputs["b2"], dtype=np.float32)
    clf_w = np.asarray(inputs["clf_w"], dtype=np.float32)
    clf_b = np.asarray(inputs["clf_b"], dtype=np.float32)
    gw1 = np.asarray(inputs["gate_w1"], dtype=np.float32)
    gb1 = np.asarray(inputs["gate_b1"], dtype=np.float32)
    gw2 = np.asarray(inputs["gate_w2"], dtype=np.float32)
    gb2 = np.asarray(inputs["gate_b2"], dtype=np.float32)

    xT16 = np.ascontiguousarray(x.T).astype(np.float16)          # [D, B]
    clf16 = clf_w.astype(np.float16)
    gw1_16 = gw1.astype(np.float16)
    gw2_16 = gw2.astype(np.float16)
    clfb_r = np.ascontiguousarray(clf_b.reshape(C // P, P).T)    # [128, 8]
    gb1_r = np.ascontiguousarray(gb1.reshape(D // P, P).T)       # [128, 8]
    gb2_b = np.ascontiguousarray(np.broadcast_to(gb2, (P, E)))   # [128, 8]

    in_maps = []
    for e in range(NCORES):
        in_maps.append(
            {
                "xT": xT16,
                "w1": W1[e].astype(np.float16),
                "w2": W2[e].astype(np.float16),
                "clf": clf16,
                "gw1": gw1_16,
                "gw2": gw2_16,
                "xg": np.ascontiguousarray(xT16[:, e * BT : (e + 1) * BT]),
                "b1": np.ascontiguousarray(b1[e].reshape(H // P, P).T),
                "b2": np.ascontiguousarray(b2[e].reshape(H // P, P).T),
                "clfb": clfb_r,
                "gb1": gb1_r,
                "gb2": gb2_b,
            }
        )

    if _CACHED_NC is None:
        _CACHED_NC = _build_nc()
    nc = _CACHED_NC

    n_time = int(os.environ.get("MOE_TIME_ITERS", "0"))
    if n_time > 0:
        results, times = _run_spmd_timed(nc, in_maps, NCORES, n_time)
        LAST_RESULT = {"results": results, "times": times}
    else:
        res = run_bass_kernel_spmd(nc, in_maps, core_ids=list(range(NCORES)))
        results = res.results
        LAST_RESULT = {"results": results, "times": None}

    logits = np.empty((B, E, C), dtype=np.float32)
    scores = np.empty((B, E), dtype=np.float32)
    for e in range(NCORES):
        logits[:, e, :] = results[e]["logits"].T
        scores[e * BT : (e + 1) * BT, :] = results[e]["scores"]
    return logits, scores
